# revision 6
# baseline (speedup 1.0000x reference)
"""CrossAttention Trainium2 kernel (batch-parallel over 8 NeuronCores).

Math (per batch element b):
    q  = Wq  @ xq + bq            [C, N]      (C=256, N=56*56=3136)
    kv = Wkv @ xkv + bkv; k, v = split(kv)
    S[n, m]  = q[:, n] . k[:, m]
    denom[m] = ||q[:, m]|| * ||k[:, m]|| + eps      (torch-broadcast quirk:
               divides along the LAST axis m, same index for both norms)
    A = softmax(S / denom, axis=m)
    out = Wproj @ (A @ v^T)^T + bproj  -> reshape + x_q residual

Device mapping (one batch element per core):
  * Everything is computed transposed where it helps:
      S^T[m, n] tiles (m on partitions) make 1/denom[m] a native per-partition
      activation scale, so exp(S*scale) is ONE fused ACT op per tile.
      |S/denom| <= 1 by Cauchy-Schwarz, so softmax needs no max-subtraction.
  * Wproj is folded into v on the host: pv = (Wproj @ Wv) @ xkv. The AV matmul
    then directly produces projected outputs; bias terms fold to
    bo = Wproj @ bv + bproj added at the end (softmax rows sum to 1).
  * AV uses an augmented pv^T|1 moving operand so the softmax row-sum arrives
    as output channel 256 of the same matmuls (no separate reduction).
  * All matmuls run in float32r (TF32-like, full PE rate); producers round
    explicitly (DVE/GPSIMD copies) as the ISA requires.
  * Column norms ||q[:, n]|| are computed from transposed projection tiles
    (qT = xq^T WqT) with ACT Square+accum_out (free-axis reduction).
"""

import sys

if "/opt/trn_rl_repo" not in sys.path:
    sys.path.insert(0, "/opt/trn_rl_repo")

import numpy as np

import concourse.bass as bass
import concourse.mybir as mybir
import concourse.tile as tile
from concourse import bacc
from concourse.bass_utils import run_bass_kernel_spmd
from concourse.masks import make_identity
from contextlib import ExitStack

F32 = mybir.dt.float32
F32R = mybir.dt.float32r
AF = mybir.ActivationFunctionType

P = 128
C = 256
CC = C // P          # 2 channel chunks
N = 56 * 56          # 3136
EPS = 1e-6
NT = 512             # free-dim tile for S^T / projections
N_TILES = [(i, min(NT, N - i)) for i in range(0, N, NT)]          # 7 tiles
M_CHUNKS = [(i, min(P, N - i)) for i in range(0, N, P)]           # 25 chunks


def _mm(nc, out, lhsT, rhs, start, stop):
    nc.tensor.matmul(out, lhsT, rhs, start=start, stop=stop)


def build(use_bias: bool, bench_reps: int = 0):
    nc = bacc.Bacc(None, target_bir_lowering=False)

    xq_d = nc.dram_tensor("xq", [C, N], F32, kind="ExternalInput")
    xkv_d = nc.dram_tensor("xkv", [C, N], F32, kind="ExternalInput")
    wq_d = nc.dram_tensor("wqT", [C, C], F32, kind="ExternalInput")   # Wq.T
    wk_d = nc.dram_tensor("wkT", [C, C], F32, kind="ExternalInput")   # Wk.T
    w3_d = nc.dram_tensor("w3T", [C, C], F32, kind="ExternalInput")   # (Wproj@Wv).T
    bq_d = nc.dram_tensor("bq", [C], F32, kind="ExternalInput")
    bk_d = nc.dram_tensor("bk", [C], F32, kind="ExternalInput")
    bo_d = nc.dram_tensor("bo", [C], F32, kind="ExternalInput")       # Wproj@bv+bproj
    out_d = nc.dram_tensor("out", [C, N], F32, kind="ExternalOutput")

    xq_v = xq_d[:].rearrange("(cc p) n -> p cc n", p=P)
    xkv_v = xkv_d[:].rearrange("(cc p) n -> p cc n", p=P)
    out_v = out_d[:].rearrange("(cc p) n -> p cc n", p=P)

    with tile.TileContext(nc) as tc, ExitStack() as ctx:
        # ---------- persistent pools ----------
        pers = ctx.enter_context(tc.tile_pool(name="pers", bufs=1))
        small = ctx.enter_context(tc.tile_pool(name="small", bufs=2))
        mm512 = ctx.enter_context(tc.tile_pool(name="mm512", bufs=2, space="PSUM"))
        accp = ctx.enter_context(tc.tile_pool(name="accp", bufs=4, space="PSUM"))

        xq_r = pers.tile([P, CC, N], F32R)
        xkv_r = pers.tile([P, CC, N], F32R)
        q_r = pers.tile([P, CC, N], F32R)
        k_r = pers.tile([P, CC, N], F32R)
        pvT = pers.tile([P, len(M_CHUNKS), C + 2], F32R)
        wq_r = pers.tile([P, CC, C], F32R)
        wk_r = pers.tile([P, CC, C], F32R)
        w3_r = pers.tile([P, CC, C], F32R)
        ident = pers.tile([P, P], F32)
        qn2 = pers.tile([P, len(M_CHUNKS)], F32)
        kn2 = pers.tile([P, len(M_CHUNKS)], F32)
        rd = pers.tile([P, len(M_CHUNKS)], F32)
        bq_sb = pers.tile([P, CC], F32)
        bk_sb = pers.tile([P, CC], F32)
        bo_sb = pers.tile([P, CC], F32)
        if use_bias:
            bqb = pers.tile([P, C], F32)
            bkb = pers.tile([P, C], F32)

        make_identity(nc, ident)
        nc.vector.memset(qn2, 1.0)
        nc.vector.memset(kn2, 1.0)
        nc.sync.dma_start(bq_sb, bq_d[:].rearrange("(c p) -> p c", p=P))
        nc.sync.dma_start(bk_sb, bk_d[:].rearrange("(c p) -> p c", p=P))
        nc.sync.dma_start(bo_sb, bo_d[:].rearrange("(c p) -> p c", p=P))
        if use_bias:
            nc.sync.dma_start(
                bqb, bass.AP(tensor=bq_d[:].tensor, offset=0, ap=[[0, P], [1, C]])
            )
            nc.sync.dma_start(
                bkb, bass.AP(tensor=bk_d[:].tensor, offset=0, ap=[[0, P], [1, C]])
            )

        # ---------- staging pool (released before the attention loop) ----------
        with tc.tile_pool(name="stage", bufs=2) as stage:
            wstg = stage.tile([P, CC, C], F32, tag="wstg", bufs=3)
            nc.sync.dma_start(wstg, wq_d[:].rearrange("(cc p) d -> p cc d", p=P))
            nc.vector.tensor_copy(wq_r, wstg)
            wstg2 = stage.tile([P, CC, C], F32, tag="wstg", bufs=3)
            nc.sync.dma_start(wstg2, wk_d[:].rearrange("(cc p) d -> p cc d", p=P))
            nc.vector.tensor_copy(wk_r, wstg2)
            wstg3 = stage.tile([P, CC, C], F32, tag="wstg", bufs=3)
            nc.sync.dma_start(wstg3, w3_d[:].rearrange("(cc p) d -> p cc d", p=P))
            nc.vector.tensor_copy(w3_r, wstg3)

            ones_f = stage.tile([P, 1], F32, tag="ones")
            nc.vector.memset(ones_f, 1.0)
            # ones column of every pv^T chunk (softmax denominator channel)
            nc.vector.tensor_copy(
                pvT[:, :, C : C + 2], ones_f.broadcast_to([P, len(M_CHUNKS), 2])
            )

            xstg = stage.tile([P, CC, N], F32, tag="xstg")
            nc.sync.dma_start(xstg, xq_v)
            nc.gpsimd.tensor_copy(xq_r, xstg)
            xstg2 = stage.tile([P, CC, N], F32, tag="xstg")
            nc.sync.dma_start(xstg2, xkv_v)
            nc.vector.tensor_copy(xkv_r, xstg2)

            # ---- q, k: [C, N] channel projections (stay in this scope) ----
            for dst, w, b, x in (
                (q_r, wq_r, bq_sb, xq_r),
                (k_r, wk_r, bk_sb, xkv_r),
            ):
                for dc in range(CC):
                    for n0, nw in N_TILES:
                        ps = mm512.tile([P, NT], F32, tag="mm512")
                        for cc in range(CC):
                            _mm(nc, ps[:, :nw], w[:, cc, dc * P : (dc + 1) * P],
                                x[:, cc, n0 : n0 + nw], cc == 0, cc == CC - 1)
                        if use_bias:
                            nc.vector.tensor_scalar_add(
                                dst[:, dc, n0 : n0 + nw], ps[:, :nw],
                                b[:, dc : dc + 1])
                        else:
                            nc.vector.tensor_copy(dst[:, dc, n0 : n0 + nw],
                                                  ps[:, :nw])

            # ---- column norms via transposed projections ----
            for w, b_bcast, nsq, x in (
                (wq_r, bqb if use_bias else None, qn2, xq_r),
                (wk_r, bkb if use_bias else None, kn2, xkv_r),
            ):
                for mi, (m0, mw) in enumerate(M_CHUNKS):
                    ps = mm512.tile([P, C], F32, tag="mm512")
                    for cc in range(CC):
                        _mm(nc, ps[:mw], x[:, cc, m0 : m0 + mw], w[:, cc, :],
                            cc == 0, cc == CC - 1)
                    scr = small.tile([P, C], F32, tag="sq", bufs=3)
                    if use_bias:
                        nc.vector.tensor_add(scr[:mw], ps[:mw], b_bcast[:mw])
                        nc.scalar.activation(scr[:mw], scr[:mw], AF.Square,
                                             accum_out=nsq[:mw, mi : mi + 1])
                    else:
                        nc.scalar.activation(scr[:mw], ps[:mw], AF.Square,
                                             accum_out=nsq[:mw, mi : mi + 1])

            # rd = 1 / (sqrt(qn2 * kn2) + eps)
            nm = len(M_CHUNKS)
            t0 = stage.tile([P, nm], F32, tag="dn")
            nc.vector.tensor_mul(t0, qn2, kn2)
            nc.scalar.activation(t0, t0, AF.Sqrt)
            nc.vector.tensor_scalar_add(t0, t0, EPS)
            nc.vector.reciprocal(rd, t0)

            # ---- pv^T chunks: (Wproj @ v)^T with m on partitions ----
            for mi, (m0, mw) in enumerate(M_CHUNKS):
                ps = mm512.tile([P, C], F32, tag="mm512")
                for cc in range(CC):
                    _mm(nc, ps[:mw], xkv_r[:, cc, m0 : m0 + mw], w3_r[:, cc, :],
                        cc == 0, cc == CC - 1)
                nc.vector.tensor_copy(pvT[:mw, mi, :C], ps[:mw])

        # ---------- late pools (reuse released staging space) ----------
        e32p = ctx.enter_context(tc.tile_pool(name="e32p", bufs=3))
        erp = ctx.enter_context(tc.tile_pool(name="erp", bufs=3))
        unp = ctx.enter_context(tc.tile_pool(name="unp", bufs=4))
        obp = ctx.enter_context(tc.tile_pool(name="obp", bufs=6))
        rcp = ctx.enter_context(tc.tile_pool(name="rcp", bufs=4))
        tpp = ctx.enter_context(tc.tile_pool(name="tpp", bufs=1, space="PSUM"))

        # ---------- attention main loop ----------
        for n0, nw in N_TILES:
            nsub = (nw + P - 1) // P
            accs = [accp.tile([P, C + 2], F32, tag="acc", name=f"acc{n0}_{s}")
                    for s in range(nsub)]
            n_mc = len(M_CHUNKS)
            for mi, (m0, mw) in enumerate(M_CHUNKS):
                sps = mm512.tile([P, NT], F32, tag="mm512")
                for cc in range(CC):
                    _mm(nc, sps[:mw, :nw], k_r[:, cc, m0 : m0 + mw],
                        q_r[:, cc, n0 : n0 + nw], cc == 0, cc == CC - 1)
                e32 = e32p.tile([P, NT], F32, tag="e32")
                nc.scalar.activation(e32[:mw, :nw], sps[:mw, :nw], AF.Exp,
                                     scale=rd[:mw, mi : mi + 1])
                er = erp.tile([P, NT], F32R, tag="er")
                nc.vector.tensor_copy(er[:mw, :nw], e32[:mw, :nw])
                for s in range(nsub):
                    bw = min(P, nw - s * P)
                    _mm(nc, accs[s][:bw], er[:mw, s * P : s * P + bw],
                        pvT[:mw, mi, :], mi == 0, mi == n_mc - 1)
            for s in range(nsub):
                bw = min(P, nw - s * P)
                rc = rcp.tile([P, 1], F32, tag="rc")
                nc.vector.reciprocal(rc[:bw], accs[s][:bw, C : C + 1])
                un = unp.tile([P, C], F32, tag="un")
                nc.vector.tensor_scalar_mul(un[:bw], accs[s][:bw, :C], rc[:bw])
                for cb in range(CC):
                    tp = tpp.tile([P, P], F32, tag="tp", bufs=2)
                    nc.tensor.transpose(tp[:, :bw], un[:bw, cb * P : (cb + 1) * P],
                                        ident[:bw, :bw])
                    ob = obp.tile([P, P], F32, tag="ob")
                    pos = n0 + s * P
                    # + residual (x_q) and output bias
                    nc.vector.tensor_add(ob[:, :bw], tp[:, :bw],
                                         xq_r[:, cb, pos : pos + bw])
                    if use_bias:
                        nc.vector.tensor_scalar_add(ob[:, :bw], ob[:, :bw],
                                                    bo_sb[:, cb : cb + 1])
                    nc.sync.dma_start(out_v[:, cb, pos : pos + bw], ob[:, :bw])

    return nc


_CACHE = {}


def _get_module(use_bias: bool):
    key = use_bias
    if key not in _CACHE:
        nc = build(use_bias)
        nc.finalize()
        _CACHE[key] = nc
    return _CACHE[key]


def kernel(x_q, x_kv, Wq, bq, Wkv, bkv, Wproj, bproj):
    x_q = np.asarray(x_q, dtype=np.float32)
    x_kv = np.asarray(x_kv, dtype=np.float32)
    Wq = np.asarray(Wq, dtype=np.float32)
    bq = np.asarray(bq, dtype=np.float32)
    Wkv = np.asarray(Wkv, dtype=np.float32)
    bkv = np.asarray(bkv, dtype=np.float32)
    Wproj = np.asarray(Wproj, dtype=np.float32)
    bproj = np.asarray(bproj, dtype=np.float32)

    B, c, H, W = x_q.shape
    assert (c, H * W) == (C, N), (x_q.shape,)
    xq = np.ascontiguousarray(x_q.reshape(B, C, N))
    xkv = np.ascontiguousarray(x_kv.reshape(B, C, N))

    Wk = Wkv[:C]
    Wv = Wkv[C:]
    wqT = np.ascontiguousarray(Wq.T)
    wkT = np.ascontiguousarray(Wk.T)
    w3T = np.ascontiguousarray((Wproj @ Wv).T)
    bk = np.ascontiguousarray(bkv[:C])
    bo = np.ascontiguousarray(Wproj @ bkv[C:] + bproj)

    use_bias = bool(np.any(bq) or np.any(bk) or np.any(bo))
    nc = _get_module(use_bias)

    in_maps = [
        {
            "xq": xq[b],
            "xkv": xkv[b],
            "wqT": wqT,
            "wkT": wkT,
            "w3T": w3T,
            "bq": bq,
            "bk": bk,
            "bo": bo,
        }
        for b in range(B)
    ]
    res = run_bass_kernel_spmd(nc, in_maps, core_ids=list(range(B)))
    out = np.stack([res.results[b]["out"] for b in range(B)], axis=0)
    return out.reshape(B, C, H, W)
